# revision 10
# baseline (speedup 1.0000x reference)
"""Depthwise 1d (per-channel linear) Trainium2 Bass kernel.

out[n, c, o] = sum_i x[n, c, i] * W[c, o, i] + b[c, o]
  x: [4096, 256, 64] f32, W: [256, 128, 64] f32, b: [256, 128] f32
  out: [4096, 256, 128] f32

Strategy: shard channels across 8 cores (32 channels/core, all 4096 rows).
The kernel is HBM-bound, so both streams move as fp16 (ample for the
2e-2 gate): x is cast + pre-transposed on the host to [pair, (2ch x 64i),
n] so the device does no transposes at all, and the output leaves the
device as fp16 [c, o, n] which the host casts/transposes back to f32.
Steady state moves only 16.8 MB in + 33.6 MB out per core, half the
fp32 traffic.

Per channel: out.T[o, n] = (W_c.T).T @ (x_c.T) -- the tiny W_c.T [64, 128]
is the PE-stationary operand (loaded once per 8 chunk-matmuls) and x
streams as the moving operand in 512-column PSUM chunks.  Evacuation
fuses the bias add: PSUM fp32 -> SBUF fp16 with the per-partition bias
column b[c, :] applied on ScalarE (Identity+bias) for half the chunks
and VectorE (tensor_scalar_add) for the other half, so no separate
bias pass exists.  x loads ride the sync HWDGE ring, output stores the
ACT ring; all DMAs move 1 MB blocks with 8 KB contiguous rows.
"""

import os

# recover cleanly if a previous run left the NeuronCores wedged; must be
# set before the runtime initializes
os.environ.setdefault("NEURON_RT_RESET_CORES", "1")

import numpy as np

import concourse.bass as bass
import concourse.tile as tile
from concourse import bacc, mybir
from concourse.bass_utils import run_bass_kernel_spmd

N_CORES = 8
N, C, HI, HO = 4096, 256, 64, 128
CLOC = C // N_CORES   # 32 channels per core
PAIRS = CLOC // 2     # 16 x-tiles of 128 partitions (2 channels each)
NCH = 512             # n-chunk per matmul == one PSUM bank of fp32
NCHUNKS = N // NCH    # 8

F32 = mybir.dt.float32
F16 = mybir.dt.float16


def build(n_cores=N_CORES):
    nc = bacc.Bacc(
        "TRN2", target_bir_lowering=False, debug=False, num_devices=n_cores
    )
    # x.T per pair: partition p = (channel 2j+p//64, feature p%64), free = n
    x_d = nc.dram_tensor("x", [PAIRS, 128, N], F16, kind="ExternalInput").ap()
    # W.T duplicated across both partition halves (PE needs lhsT and rhs
    # at the same base partition): wt[64*h + i, c, o] = W[c, o, i]
    w_d = nc.dram_tensor("wt", [128, CLOC, HO], F16, kind="ExternalInput").ap()
    # b.T: bt[o, c] = b[c, o]
    b_d = nc.dram_tensor("bt", [HO, CLOC], F32, kind="ExternalInput").ap()
    # transposed output: out[c, o, n]
    o_d = nc.dram_tensor("out", [CLOC, HO, N], F16, kind="ExternalOutput").ap()

    with tile.TileContext(nc) as tc:
        with (
            tc.tile_pool(name="const", bufs=1) as const,
            tc.tile_pool(name="xp", bufs=3) as xp,
            tc.tile_pool(name="op", bufs=4) as op,
            tc.tile_pool(name="psp", bufs=8, space="PSUM") as psp,
        ):
            def load_pair(p):
                t = xp.tile([128, N], F16, name=f"x{p}", tag="x")
                nc.sync.dma_start(out=t, in_=x_d[p])
                return t

            # PE warm-up: the HAM clock gate only lifts the idle 1.2 GHz
            # throttle after ~3.4 us of gapless PE activity, and the
            # steady-state matmul stream has small sync gaps that keep it
            # cold (measured 611 ns/matmul = exactly the cold rate).  A
            # dependency-free burst of dummy matmuls during the initial
            # DMA ramp flips the clock to 2.4 GHz; steady state never
            # idles >3.4 us so it stays warm.
            wdum = const.tile([HI, HO], F16, tag="wdum")
            nc.vector.memset(wdum, 0.0)
            xdum = const.tile([HI, NCH], F16, tag="xdum")
            nc.vector.memset(xdum, 0.0)

            # first x tiles go out before the constants so the DMA
            # engines ramp on the bulk stream immediately
            x_tiles = [load_pair(0), load_pair(1)]

            # constants ride the ACT ring (idle until the first stores) so
            # they don't serialize behind the x stream on the sync ring
            wt = const.tile([128, CLOC, HO], F16, tag="wt")
            nc.scalar.dma_start(out=wt, in_=w_d)
            bt = const.tile([HO, CLOC], F32, tag="bt")
            nc.scalar.dma_start(out=bt, in_=b_d)
            # trigger the one-time ACT Identity table load while the
            # first x DMAs are still in flight
            warm = const.tile([HO, 1], F32, tag="warm")
            nc.scalar.add(out=warm, in_=bt[:, 0:1], add=bt[:, 1:2])

            for _ in range(12):
                pswu = psp.tile([HO, NCH], F32, tag="ps")
                nc.tensor.matmul(pswu, lhsT=wdum, rhs=xdum, start=True, stop=True)

            def emit_pair(p, x_sb):
                for ci in range(2):
                    c = 2 * p + ci
                    o_sb = op.tile([HO, N], F16, name=f"o{c}", tag="o")
                    for k in range(NCHUNKS):
                        ps = psp.tile([HO, NCH], F32, tag="ps")
                        nc.tensor.matmul(
                            ps,
                            lhsT=wt[64 * ci : 64 * ci + 64, c, :],
                            rhs=x_sb[64 * ci : 64 * ci + 64, k * NCH : (k + 1) * NCH],
                            start=True,
                            stop=True,
                        )
                        # fused bias + fp16 cast on PSUM evacuation,
                        # alternating engines to split the load
                        sl = o_sb[:, k * NCH : (k + 1) * NCH]
                        if (k + ci) % 2 == 0:
                            nc.scalar.add(out=sl, in_=ps, add=bt[:, c : c + 1])
                        else:
                            nc.vector.tensor_scalar_add(sl, ps, bt[:, c : c + 1])
                    nc.scalar.dma_start(out=o_d[c], in_=o_sb)

            for p in range(PAIRS):
                # keep the x stream two tiles ahead of compute
                if p + 2 < PAIRS:
                    x_tiles.append(load_pair(p + 2))
                emit_pair(p, x_tiles[p])
    nc.compile()
    return nc


def make_in_maps(x, W, b):
    xh = np.asarray(x, dtype=np.float32).astype(np.float16)
    # [n, core, pair, ci, i] -> [core, pair, (ci, i), n]
    xt = np.ascontiguousarray(
        xh.reshape(N, N_CORES, PAIRS, 2, HI).transpose(1, 2, 3, 4, 0)
    ).reshape(N_CORES, PAIRS, 128, N)
    Wh = np.asarray(W, dtype=np.float32).astype(np.float16)
    wt1 = Wh.reshape(N_CORES, CLOC, HO, HI).transpose(0, 3, 1, 2)  # [core, i, c, o]
    wts = np.ascontiguousarray(
        np.concatenate([wt1, wt1], axis=1)
    )  # [core, 2*64 i, c, o] (duplicated halves)
    bb = np.asarray(b, dtype=np.float32)
    bts = np.ascontiguousarray(
        bb.reshape(N_CORES, CLOC, HO).transpose(0, 2, 1)
    )  # [core, o, c]
    return [
        {"x": xt[i], "wt": wts[i], "bt": bts[i]}
        for i in range(N_CORES)
    ]


def assemble_out(results):
    final = np.empty((N, C, HO), dtype=np.float32)
    fv = final.transpose(1, 2, 0)  # [C, HO, N] view of final
    for i in range(N_CORES):
        fv[i * CLOC : (i + 1) * CLOC] = results[i]["out"]
    return final


_cache = {}


def kernel(x, W, b):
    nc = _cache.get("nc")
    if nc is None:
        nc = _cache["nc"] = build()
    in_maps = make_in_maps(x, W, b)
    res = run_bass_kernel_spmd(nc, in_maps, core_ids=list(range(N_CORES)))
    return assemble_out(res.results)


# revision 12
# speedup vs baseline: 1.1467x; 1.1467x over previous
"""Depthwise 1d (per-channel linear) Trainium2 Bass kernel.

out[n, c, o] = sum_i x[n, c, i] * W[c, o, i] + b[c, o]
  x: [4096, 256, 64] f32, W: [256, 128, 64] f32, b: [256, 128] f32
  out: [4096, 256, 128] f32

Strategy: shard channels across 8 cores (32 channels/core, all 4096 rows).
The kernel is HBM-bound, so both streams move as fp16 (ample for the
2e-2 gate): x is cast + pre-transposed on the host to [pair, (2ch x 64i),
n] so the device does no transposes at all, and the output leaves the
device as fp16 which the host casts/transposes back to f32.  Steady
state moves only 16.8 MB in + 33.6 MB out per core, half the fp32
traffic.

Compute: channels are processed in pairs with a block-diagonal
PE-stationary operand so the contraction uses the full K=128 array
(K=64 single-channel matmuls keep the PE HAM clock gate throttled at
1.2 GHz; full-array work runs at 2.4 GHz once warmed by the dummy
burst below).  Each matmul covers one o-half of both channels:
lhsT[128, 128] = diag(W_c0[oh].T, W_c1[oh].T), rhs = the pair's x tile
[128, 512-col n-chunk], out partitions = (c0 o-half | c1 o-half).
Evacuation fuses the bias add and fp16 cast: PSUM fp32 -> SBUF fp16
with a per-partition bias column, on ScalarE (Identity+bias) for half
the chunks and VectorE (tensor_scalar_add) for the rest.  x loads ride
the sync HWDGE ring, stores + constants the ACT ring; all bulk DMAs
move 1 MB blocks with 8 KB contiguous rows.  The host un-interleaves
the [pair, o-half, (c0|c1), n] device layout during the f32 upcast.
"""

import os

# recover cleanly if a previous run left the NeuronCores wedged; must be
# set before the runtime initializes
os.environ.setdefault("NEURON_RT_RESET_CORES", "1")

import numpy as np

import concourse.bass as bass
import concourse.tile as tile
from concourse import bacc, mybir
from concourse.bass_utils import run_bass_kernel_spmd

N_CORES = 8
N, C, HI, HO = 4096, 256, 64, 128
CLOC = C // N_CORES   # 32 channels per core
PAIRS = CLOC // 2     # 16 x-tiles of 128 partitions (2 channels each)
NCH = 512             # n-chunk per matmul == one PSUM bank of fp32
NCHUNKS = N // NCH    # 8

F32 = mybir.dt.float32
F16 = mybir.dt.float16


def build(n_cores=N_CORES):
    nc = bacc.Bacc(
        "TRN2", target_bir_lowering=False, debug=False, num_devices=n_cores
    )
    # x.T per pair: partition p = (channel 2j + p//64, feature p%64), free = n
    x_d = nc.dram_tensor("x", [PAIRS, 128, N], F16, kind="ExternalInput").ap()
    # block-diagonal stationary: wbd[i, p, h, m] =
    #   W[2p,   64h + m,      i]       for i < 64, m < 64
    #   W[2p+1, 64h + m - 64, i - 64]  for i >= 64, m >= 64, else 0
    w_d = nc.dram_tensor("wbd", [128, PAIRS, 2, 128], F16, kind="ExternalInput").ap()
    # bias columns matching the matmul output partitions
    b_d = nc.dram_tensor("bt2", [128, PAIRS, 2], F32, kind="ExternalInput").ap()
    # transposed output: out[p, h, (c0 o-half | c1 o-half), n]
    o_d = nc.dram_tensor("out", [PAIRS, 2, 128, N], F16, kind="ExternalOutput").ap()

    with tile.TileContext(nc) as tc:
        with (
            tc.tile_pool(name="const", bufs=1) as const,
            tc.tile_pool(name="xp", bufs=4) as xp,
            tc.tile_pool(name="op", bufs=4) as op,
            tc.tile_pool(name="psp", bufs=8, space="PSUM") as psp,
        ):
            def load_pair(p):
                t = xp.tile([128, N], F16, name=f"x{p}", tag="x")
                nc.sync.dma_start(out=t, in_=x_d[p])
                return t

            # PE warm-up: the HAM clock gate keeps the PE at 1.2 GHz until
            # it sees a sustained window of full-array activity; a
            # dependency-free dummy burst during the initial DMA ramp
            # flips it to 2.4 GHz before the real stream begins.
            wdum = const.tile([128, HO], F16, tag="wdum")
            nc.vector.memset(wdum, 0.0)
            xdum = const.tile([128, NCH], F16, tag="xdum")
            nc.vector.memset(xdum, 0.0)

            # first x tiles go out before the constants so the DMA
            # engines ramp on the bulk stream immediately
            x_tiles = [load_pair(0), load_pair(1), load_pair(2)]

            # constants ride the ACT ring (idle until the first stores) so
            # they don't serialize behind the x stream on the sync ring
            wbd = const.tile([128, PAIRS, 2, 128], F16, tag="wbd")
            nc.scalar.dma_start(out=wbd, in_=w_d)
            bt2 = const.tile([128, PAIRS, 2], F32, tag="bt2")
            nc.scalar.dma_start(out=bt2, in_=b_d)
            # trigger the one-time ACT Identity table load while the
            # first x DMAs are still in flight
            warm = const.tile([128, 1], F32, tag="warm")
            nc.scalar.add(out=warm, in_=bt2[:, 0, 0:1], add=bt2[:, 0, 1:2])

            for _ in range(18):
                pswu = psp.tile([128, NCH], F32, tag="ps")
                nc.tensor.matmul(pswu, lhsT=wdum, rhs=xdum, start=True, stop=True)

            def emit_pair(p, x_sb):
                for h in range(2):
                    o_sb = op.tile([128, N], F16, name=f"o{p}_{h}", tag="o")
                    for k in range(NCHUNKS):
                        ps = psp.tile([128, NCH], F32, tag="ps")
                        nc.tensor.matmul(
                            ps,
                            lhsT=wbd[:, p, h, :],
                            rhs=x_sb[:, k * NCH : (k + 1) * NCH],
                            start=True,
                            stop=True,
                        )
                        # fused bias + fp16 cast on PSUM evacuation,
                        # alternating engines to split the load
                        sl = o_sb[:, k * NCH : (k + 1) * NCH]
                        if (k + h) % 2 == 0:
                            nc.scalar.add(out=sl, in_=ps, add=bt2[:, p, h : h + 1])
                        else:
                            nc.vector.tensor_scalar_add(sl, ps, bt2[:, p, h : h + 1])
                    nc.scalar.dma_start(out=o_d[p, h], in_=o_sb)

            for p in range(PAIRS):
                # keep the x stream three tiles ahead of compute
                if p + 3 < PAIRS:
                    x_tiles.append(load_pair(p + 3))
                emit_pair(p, x_tiles[p])
    nc.compile()
    return nc


def make_in_maps(x, W, b):
    xh = np.asarray(x, dtype=np.float32).astype(np.float16)
    # [n, core, pair, ci, i] -> [core, pair, (ci, i), n]
    xt = np.ascontiguousarray(
        xh.reshape(N, N_CORES, PAIRS, 2, HI).transpose(1, 2, 3, 4, 0)
    ).reshape(N_CORES, PAIRS, 128, N)
    Wh = np.asarray(W, dtype=np.float32).astype(np.float16)
    # [core, pair, ci, h, o_loc, i]
    Wr = Wh.reshape(N_CORES, PAIRS, 2, 2, 64, HI)
    wbd = np.zeros((N_CORES, 128, PAIRS, 2, 128), dtype=np.float16)
    # [core, i, pair, h, o_loc]
    wbd[:, 0:64, :, :, 0:64] = Wr[:, :, 0].transpose(0, 4, 1, 2, 3)
    wbd[:, 64:128, :, :, 64:128] = Wr[:, :, 1].transpose(0, 4, 1, 2, 3)
    bb = np.asarray(b, dtype=np.float32)
    br = bb.reshape(N_CORES, PAIRS, 2, 2, 64)  # [core, pair, ci, h, o_loc]
    bt2 = np.empty((N_CORES, 128, PAIRS, 2), dtype=np.float32)
    bt2[:, 0:64] = br[:, :, 0].transpose(0, 3, 1, 2)
    bt2[:, 64:128] = br[:, :, 1].transpose(0, 3, 1, 2)
    return [
        {"x": xt[i], "wbd": wbd[i], "bt2": bt2[i]}
        for i in range(N_CORES)
    ]


def assemble_out(results):
    final = np.empty((N, C, HO), dtype=np.float32)
    fv = final.transpose(1, 2, 0)  # [C, HO, N] view of final
    for i in range(N_CORES):
        dev = results[i]["out"]  # [PAIRS, 2, 128, N] fp16
        r = dev.reshape(PAIRS, 2, 2, 64, N)  # [p, h, ci, o_loc, n]
        fv[i * CLOC : (i + 1) * CLOC] = r.transpose(0, 2, 1, 3, 4).reshape(
            CLOC, HO, N
        )
    return final


_cache = {}


def kernel(x, W, b):
    nc = _cache.get("nc")
    if nc is None:
        nc = _cache["nc"] = build()
    in_maps = make_in_maps(x, W, b)
    res = run_bass_kernel_spmd(nc, in_maps, core_ids=list(range(N_CORES)))
    return assemble_out(res.results)


# revision 14
# speedup vs baseline: 1.1693x; 1.0197x over previous
"""Depthwise 1d (per-channel linear) Trainium2 Bass kernel.

out[n, c, o] = sum_i x[n, c, i] * W[c, o, i] + b[c, o]
  x: [4096, 256, 64] f32, W: [256, 128, 64] f32, b: [256, 128] f32
  out: [4096, 256, 128] f32

Strategy: shard channels across 8 cores (32 channels/core, all 4096 rows).
The kernel is HBM-bound, so both streams move as fp16 (ample for the
2e-2 gate): x is cast + pre-transposed on the host to [pair, (2ch x 64i),
n] so the device does no transposes at all, and the output leaves the
device as fp16 which the host casts/transposes back to f32.  Steady
state moves only 16.8 MB in + 33.6 MB out per core, half the fp32
traffic.

Compute: channels are processed in pairs with a block-diagonal
PE-stationary operand so the contraction uses the full K=128 array
(K=64 single-channel matmuls keep the PE HAM clock gate throttled at
1.2 GHz; full-array work runs at 2.4 GHz once warmed by the dummy
burst below).  Each matmul covers one o-half of both channels:
lhsT[128, 128] = diag(W_c0[oh].T, W_c1[oh].T), rhs = the pair's x tile
[128, 512-col n-chunk], out partitions = (c0 o-half | c1 o-half).
Evacuation fuses the bias add and fp16 cast: PSUM fp32 -> SBUF fp16
with a per-partition bias column, on ScalarE (Identity+bias) for half
the chunks and VectorE (tensor_scalar_add) for the rest.  x loads ride
the sync HWDGE ring, stores + constants the ACT ring; all bulk DMAs
move 1 MB blocks with 8 KB contiguous rows.  The host un-interleaves
the [pair, o-half, (c0|c1), n] device layout during the f32 upcast.
"""

import os

# recover cleanly if a previous run left the NeuronCores wedged; must be
# set before the runtime initializes
os.environ.setdefault("NEURON_RT_RESET_CORES", "1")

import numpy as np

import concourse.bass as bass
import concourse.tile as tile
from concourse import bacc, mybir
from concourse.bass_utils import run_bass_kernel_spmd

N_CORES = 8
N, C, HI, HO = 4096, 256, 64, 128
CLOC = C // N_CORES   # 32 channels per core
PAIRS = CLOC // 2     # 16 x-tiles of 128 partitions (2 channels each)
NCH = 512             # n-chunk per matmul == one PSUM bank of fp32
NCHUNKS = N // NCH    # 8

F32 = mybir.dt.float32
F16 = mybir.dt.float16


def build(n_cores=N_CORES):
    nc = bacc.Bacc(
        "TRN2", target_bir_lowering=False, debug=False, num_devices=n_cores
    )
    # x.T per pair: partition p = (channel 2j + p//64, feature p%64), free = n
    x_d = nc.dram_tensor("x", [PAIRS, 128, N], F16, kind="ExternalInput").ap()
    # block-diagonal stationary: wbd[i, p, h, m] =
    #   W[2p,   64h + m,      i]       for i < 64, m < 64
    #   W[2p+1, 64h + m - 64, i - 64]  for i >= 64, m >= 64, else 0
    w_d = nc.dram_tensor("wbd", [128, PAIRS, 2, 128], F16, kind="ExternalInput").ap()
    # bias columns matching the matmul output partitions
    b_d = nc.dram_tensor("bt2", [128, PAIRS, 2], F32, kind="ExternalInput").ap()
    # transposed output: out[p, h, (c0 o-half | c1 o-half), n]
    o_d = nc.dram_tensor("out", [PAIRS, 2, 128, N], F16, kind="ExternalOutput").ap()

    with tile.TileContext(nc) as tc:
        with (
            tc.tile_pool(name="const", bufs=1) as const,
            tc.tile_pool(name="xp", bufs=4) as xp,
            tc.tile_pool(name="op", bufs=4) as op,
            tc.tile_pool(name="psp", bufs=8, space="PSUM") as psp,
        ):
            def load_pair(p):
                t = xp.tile([128, N], F16, name=f"x{p}", tag="x")
                nc.sync.dma_start(out=t, in_=x_d[p])
                return t

            # PE warm-up: the HAM clock gate keeps the PE at 1.2 GHz until
            # it sees a sustained window of full-array activity; a
            # dependency-free dummy burst during the initial DMA ramp
            # flips it to 2.4 GHz before the real stream begins.
            wdum = const.tile([128, HO], F16, tag="wdum")
            nc.vector.memset(wdum, 0.0)
            xdum = const.tile([128, NCH], F16, tag="xdum")
            nc.vector.memset(xdum, 0.0)

            # first x tiles go out before the constants so the DMA
            # engines ramp on the bulk stream immediately
            x_tiles = [load_pair(0), load_pair(1), load_pair(2)]

            # constants ride the ACT ring (idle until the first stores) so
            # they don't serialize behind the x stream on the sync ring
            bt2 = const.tile([128, PAIRS, 2], F32, tag="bt2")
            nc.scalar.dma_start(out=bt2, in_=b_d)
            wbd = const.tile([128, PAIRS, 2, 128], F16, tag="wbd")
            nc.scalar.dma_start(out=wbd, in_=w_d)
            # trigger the one-time ACT Identity table load while the
            # first x DMAs are still in flight
            warm = const.tile([128, 1], F32, tag="warm")
            nc.scalar.add(out=warm, in_=bt2[:, 0, 0:1], add=bt2[:, 0, 1:2])

            for _ in range(18):
                pswu = psp.tile([128, NCH], F32, tag="ps")
                nc.tensor.matmul(pswu, lhsT=wdum, rhs=xdum, start=True, stop=True)

            def emit_pair(p, x_sb):
                # the final stores drain after the last evacs; splitting
                # them lets the tail overlap instead of serializing ~3 us
                # of post-compute DMA
                nsplit = 4 if p == PAIRS - 1 else 1
                for h in range(2):
                    o_sb = op.tile([128, N], F16, name=f"o{p}_{h}", tag="o")
                    for k in range(NCHUNKS):
                        ps = psp.tile([128, NCH], F32, tag="ps")
                        nc.tensor.matmul(
                            ps,
                            lhsT=wbd[:, p, h, :],
                            rhs=x_sb[:, k * NCH : (k + 1) * NCH],
                            start=True,
                            stop=True,
                        )
                        # fused bias + fp16 cast on PSUM evacuation,
                        # alternating engines to split the load
                        sl = o_sb[:, k * NCH : (k + 1) * NCH]
                        if (k + h) % 2 == 0:
                            nc.scalar.add(out=sl, in_=ps, add=bt2[:, p, h : h + 1])
                        else:
                            nc.vector.tensor_scalar_add(sl, ps, bt2[:, p, h : h + 1])
                        done = k + 1
                        if done % (NCHUNKS // nsplit) == 0:
                            c0 = (done - NCHUNKS // nsplit) * NCH
                            c1 = done * NCH
                            nc.scalar.dma_start(
                                out=o_d[p, h, :, c0:c1], in_=o_sb[:, c0:c1]
                            )

            for p in range(PAIRS):
                # keep the x stream three tiles ahead of compute
                if p + 3 < PAIRS:
                    x_tiles.append(load_pair(p + 3))
                emit_pair(p, x_tiles[p])
    nc.compile()
    return nc


def make_in_maps(x, W, b):
    xh = np.asarray(x, dtype=np.float32).astype(np.float16)
    # [n, core, pair, ci, i] -> [core, pair, (ci, i), n]
    xt = np.ascontiguousarray(
        xh.reshape(N, N_CORES, PAIRS, 2, HI).transpose(1, 2, 3, 4, 0)
    ).reshape(N_CORES, PAIRS, 128, N)
    Wh = np.asarray(W, dtype=np.float32).astype(np.float16)
    # [core, pair, ci, h, o_loc, i]
    Wr = Wh.reshape(N_CORES, PAIRS, 2, 2, 64, HI)
    wbd = np.zeros((N_CORES, 128, PAIRS, 2, 128), dtype=np.float16)
    # [core, i, pair, h, o_loc]
    wbd[:, 0:64, :, :, 0:64] = Wr[:, :, 0].transpose(0, 4, 1, 2, 3)
    wbd[:, 64:128, :, :, 64:128] = Wr[:, :, 1].transpose(0, 4, 1, 2, 3)
    bb = np.asarray(b, dtype=np.float32)
    br = bb.reshape(N_CORES, PAIRS, 2, 2, 64)  # [core, pair, ci, h, o_loc]
    bt2 = np.empty((N_CORES, 128, PAIRS, 2), dtype=np.float32)
    bt2[:, 0:64] = br[:, :, 0].transpose(0, 3, 1, 2)
    bt2[:, 64:128] = br[:, :, 1].transpose(0, 3, 1, 2)
    return [
        {"x": xt[i], "wbd": wbd[i], "bt2": bt2[i]}
        for i in range(N_CORES)
    ]


def assemble_out(results):
    final = np.empty((N, C, HO), dtype=np.float32)
    fv = final.transpose(1, 2, 0)  # [C, HO, N] view of final
    for i in range(N_CORES):
        dev = results[i]["out"]  # [PAIRS, 2, 128, N] fp16
        r = dev.reshape(PAIRS, 2, 2, 64, N)  # [p, h, ci, o_loc, n]
        fv[i * CLOC : (i + 1) * CLOC] = r.transpose(0, 2, 1, 3, 4).reshape(
            CLOC, HO, N
        )
    return final


_cache = {}


def kernel(x, W, b):
    nc = _cache.get("nc")
    if nc is None:
        nc = _cache["nc"] = build()
    in_maps = make_in_maps(x, W, b)
    res = run_bass_kernel_spmd(nc, in_maps, core_ids=list(range(N_CORES)))
    return assemble_out(res.results)
